# revision 4
# baseline (speedup 1.0000x reference)
"""Banded sparse attention + MLP projections for TRN2, 8-core SPMD.

Problem: out = (softmax(mask(Q K^T / sqrt(dk))) V) W_O + b_O with
Q/K/V = x W_{Q,K,V} + b, x:[4, 2048, 512], 8 heads, dk=64.

The "log-sparse + k neighbors" mask with k = S//2 = 1024 degenerates to a
banded causal mask: valid iff 0 <= i - j <= 1024 (powers of 2 above 1024
exceed the max distance 2047).  Each 128-query tile attends to <= 9 key
tiles.

Sharding: 8 cores = 4 batches x 2 head-groups (4 heads each).

v2 restructure (vs the 124.7us qt-major baseline):
  - kt-major QK: one stationary K^T tile per key tile, streamed against all
    in-band query columns of both heads (zero-padded per head).  Cuts score
    LDWEIGHTS from 432 to ~70 per core and score matmuls from 432 to ~220.
  - One exp per (kt, q-chunk): 52 activation calls instead of 64, each over
    a [128, 2, <=768] f32 PSUM region (3-bank slots, double buffered).
  - Band-edge masks (upper/lower triangular) run on GpSimd (SBUF-only), the
    only engine with idle capacity; exp'd scores for both heads are masked
    in one op via duplicated [128, 2, 128] mask tiles.
  - PV accumulates into a single [128, 2, 65] PSUM tile (ones-column of V
    gives the softmax denominator); one reciprocal + one broadcast multiply
    per query tile normalizes both heads at once.
  - Projections (Q/K pr0 up front + V per key tile + Q/K pr1 and the
    O-projection interleaved into the attention loops) run through 1-bank
    PSUM slots; input DMAs are issued across 4 DGE queues in column-chunk
    order so the first QK matmul starts ~3us in.

All matmuls run in bf16 (fp32 PSUM accumulation); measured end-to-end
scale-relative absmax error vs the fp32 reference is ~5e-3.
"""

import functools
from contextlib import ExitStack

import numpy as np
import ml_dtypes

import concourse.bacc as bacc
import concourse.bass as bass
import concourse.mybir as mybir
import concourse.tile as tile
from concourse.bass_utils import run_bass_kernel_spmd
from concourse.masks import make_identity, make_upper_triangular, make_lower_triangular

BF16 = mybir.dt.bfloat16
F32 = mybir.dt.float32
NBF = ml_dtypes.bfloat16

S, D = 2048, 512
NT = S // 128          # 16 token tiles
MAXNK = 9              # max key tiles in the band per query tile
N_CORES = 8

LAST_RESULTS = None    # BassKernelResults of the most recent run (for profiling)


def _qk_subchunks(h2, ncols):
    """Split [0, ncols) score columns so each matmul output stays inside one
    PSUM bank.  The [128, 2, 768] f32 score tile is bank-aligned; the h2=1
    block starts at byte 3072, so its bank boundary falls at column 256."""
    cuts = (512,) if h2 == 0 else (256,)
    edges = [0] + [c for c in cuts if c < ncols] + [ncols]
    return list(zip(edges[:-1], edges[1:]))


def _emit(ctx: ExitStack, tc, io, use_bias):
    nc = tc.nc
    xT, wq, wk, wv, wo, bq, bk, bv, outT = (
        io[k] for k in ("xT", "wq", "wk", "wv", "wo", "bq", "bk", "bv", "outT")
    )

    persist = ctx.enter_context(tc.tile_pool(name="persist", bufs=1))

    ident = persist.tile([128, 128], BF16)
    make_identity(nc, ident)
    # scores are held transposed: [kpos (partition), q (free)].
    # diag tile (kt == qt) valid iff q >= k  -> upper triangular incl diag
    # far tile (qt == kt+8) valid iff q <= k -> lower triangular incl diag
    # masks duplicated along a middle h2 axis so one multiply covers the pair
    m_diag2 = persist.tile([128, 2, 128], BF16)
    m_left2 = persist.tile([128, 2, 128], BF16)
    for h2 in range(2):
        make_upper_triangular(nc, m_diag2[:, h2, :], val=1.0, diag=True)
        make_lower_triangular(nc, m_left2[:, h2, :], val=1.0, diag=True)
    ones_row = persist.tile([1, 512], BF16)
    nc.vector.memset(ones_row, 1.0)
    # warm up the exp table (~1.3us ACT_TABLE_LOAD) while DMAs run
    dummy = persist.tile([1, 8], F32)
    nc.vector.memset(dummy, 0.0)
    nc.scalar.activation(
        out=dummy, in_=dummy, func=mybir.ActivationFunctionType.Exp
    )

    xT_sb = persist.tile([128, 4, S], BF16)
    wq_sb = persist.tile([128, 4, 256], BF16)
    wk_sb = persist.tile([128, 4, 256], BF16)
    wv_sb = persist.tile([128, 4, 256], BF16)
    bq_sb = persist.tile([1, 256], BF16)
    bk_sb = persist.tile([1, 256], BF16)
    bv_sb = persist.tile([1, 256], BF16)
    wo_sb = persist.tile([128, 2, 512], BF16)

    # all input DMAs on the SP queue (compute-engine queues stay clear of
    # DMA-issue overhead), ordered so the first QK matmul's inputs land first
    for kt in range(4):
        nc.sync.dma_start(out=wk_sb[:, kt, :], in_=wk[kt * 128:(kt + 1) * 128, :])
    for ktr in range(4):
        nc.sync.dma_start(out=xT_sb[:, ktr, 0:512], in_=xT[ktr * 128:(ktr + 1) * 128, 0:512])
    for kt in range(4):
        nc.sync.dma_start(out=wq_sb[:, kt, :], in_=wq[kt * 128:(kt + 1) * 128, :])
        nc.sync.dma_start(out=wv_sb[:, kt, :], in_=wv[kt * 128:(kt + 1) * 128, :])
    for ch in (1, 2, 3):
        cs = slice(ch * 512, (ch + 1) * 512)
        for ktr in range(4):
            nc.sync.dma_start(out=xT_sb[:, ktr, cs], in_=xT[ktr * 128:(ktr + 1) * 128, cs])
    for pr in range(2):
        nc.sync.dma_start(out=wo_sb[:, pr, :], in_=wo[pr * 128:(pr + 1) * 128, :])
    if use_bias:
        nc.sync.dma_start(out=bq_sb, in_=bq[:, :])
        nc.sync.dma_start(out=bk_sb, in_=bk[:, :])
        nc.sync.dma_start(out=bv_sb, in_=bv[:, :])

    # K^T per head pair: rows 0-63 head A dims, 64-127 head B dims.
    # Q^T zero-padded per head (block = pr*2+h2): the head's 64 dim-rows hold
    # data, the other 64 rows are zero, so the QK matmul can contract the
    # full 128-row K^T pair block (FWL-eligible) with exact results.
    QT_zp = persist.tile([128, 4, S], BF16)
    KT_sb = persist.tile([128, 2, S], BF16)
    nc.gpsimd.memset(QT_zp[64:128, 0, :], 0.0)
    nc.gpsimd.memset(QT_zp[0:64, 1, :], 0.0)
    nc.gpsimd.memset(QT_zp[64:128, 2, :], 0.0)
    nc.gpsimd.memset(QT_zp[0:64, 3, :], 0.0)
    # V in [token, d] layout per k-tile, stored as [dA0..dA63, onesA,
    # dB0..dB63, onesB] so [V_h | ones] is one contiguous [128, 65] slice.
    V_sb = persist.tile([128, 2, NT, 130], BF16)
    nc.gpsimd.memset(V_sb[:, :, :, 64:65], 1.0)
    nc.gpsimd.memset(V_sb[:, :, :, 129:130], 1.0)
    # normalized attention output, transposed: rows = head dims of the pair
    OT_sb = persist.tile([128, 2, S], BF16)
    # exp'd (masked) scores for the whole band of one pair:
    # [kpos, key-tile, h2, in-band query column]
    pT = persist.tile([128, NT, 2, MAXNK * 128], BF16)

    sc = ctx.enter_context(tc.tile_pool(name="sc", bufs=2, space="PSUM"))
    sm = ctx.enter_context(tc.tile_pool(name="sm", bufs=2, space="PSUM"))
    obp = ctx.enter_context(tc.tile_pool(name="obp", bufs=4))
    rcp = ctx.enter_context(tc.tile_pool(name="rcp", bufs=4))
    fsp = ctx.enter_context(tc.tile_pool(name="fsp", bufs=4))

    def emit_qk_proj(is_q, pr, ch):
        """One 512-column chunk of the Q or K projection for pair `pr`."""
        w_sb, b_sb = (wq_sb, bq_sb) if is_q else (wk_sb, bk_sb)
        ps = sm.tile([128, 512], F32, tag="sm", name="ps_p")
        for kt in range(4):
            nc.tensor.matmul(
                ps,
                lhsT=w_sb[:, kt, pr * 128:(pr + 1) * 128],
                rhs=xT_sb[:, kt, ch * 512:(ch + 1) * 512],
                start=(kt == 0),
                stop=(kt == 3 and not use_bias),
            )
        if use_bias:
            nc.tensor.matmul(
                ps,
                lhsT=b_sb[:, pr * 128:(pr + 1) * 128],
                rhs=ones_row,
                start=False,
                stop=True,
            )
        cs = slice(ch * 512, (ch + 1) * 512)
        if is_q:
            nc.vector.tensor_copy(out=QT_zp[0:64, pr * 2, cs], in_=ps[0:64, :])
            nc.vector.tensor_copy(out=QT_zp[64:128, pr * 2 + 1, cs], in_=ps[64:128, :])
        else:
            nc.vector.tensor_copy(out=KT_sb[:, pr, cs], in_=ps)

    def emit_v(tt):
        """V projection for one token tile, interleaved into pair layout."""
        ps = sm.tile([128, 256], F32, tag="sm", name="ps_v")
        for kt in range(4):
            nc.tensor.matmul(
                ps,
                lhsT=xT_sb[:, kt, tt * 128:(tt + 1) * 128],
                rhs=wv_sb[:, kt, 0:256],
                start=(kt == 0),
                stop=(kt == 3 and not use_bias),
            )
        if use_bias:
            nc.tensor.matmul(
                ps, lhsT=ones_row[:, 0:128], rhs=bv_sb[:, 0:256],
                start=False, stop=True,
            )
        src = ps.rearrange("p (pr h2 d) -> p pr h2 d", pr=2, h2=2)
        dst = V_sb[:, :, tt, :].rearrange("p pr (h2 dd) -> p pr h2 dd", h2=2)
        nc.vector.tensor_copy(out=dst[:, :, :, 0:64], in_=src)

    def emit_scores(pr, kt):
        """All in-band scores with key tile kt: QK matmuls, exp, edge masks."""
        nq = min(MAXNK, NT - kt)
        chunks = [(0, min(nq, 6))]
        if nq > 6:
            chunks.append((6, nq - 6))
        for q0, qn in chunks:
            ps_s = sc.tile([128, 2, 768], F32, tag="sc", name="ps_s")
            for h2 in range(2):
                for s0, s1 in _qk_subchunks(h2, qn * 128):
                    c0 = (kt + q0) * 128 + s0
                    nc.tensor.matmul(
                        ps_s[:, h2, s0:s1],
                        lhsT=KT_sb[:, pr, kt * 128:(kt + 1) * 128],
                        rhs=QT_zp[:, pr * 2 + h2, c0:c0 + (s1 - s0)],
                        start=True,
                        stop=True,
                    )
            nc.scalar.activation(
                out=pT[:, kt, :, q0 * 128:(q0 + qn) * 128],
                in_=ps_s[:, :, 0:qn * 128],
                func=mybir.ActivationFunctionType.Exp,
                scale=0.125,  # 1/sqrt(dk)
            )
            if q0 == 0:
                nc.gpsimd.tensor_mul(
                    out=pT[:, kt, :, 0:128], in0=pT[:, kt, :, 0:128], in1=m_diag2
                )
            if q0 + qn == MAXNK:  # far tile (qt = kt+8) exists
                nc.gpsimd.tensor_mul(
                    out=pT[:, kt, :, 1024:1152],
                    in0=pT[:, kt, :, 1024:1152],
                    in1=m_left2,
                )

    def emit_pv(pr, qt):
        """PV accumulation + softmax normalization + transpose for one
        query tile (both heads of the pair)."""
        nk = min(qt, MAXNK - 1) + 1
        kt_lo = qt - nk + 1
        ps_o = sm.tile([128, 2, 65], F32, tag="sm", name="ps_o")
        for h2 in range(2):
            for j in range(nk):
                kt2 = kt_lo + j
                lq = (qt - kt2) * 128
                nc.tensor.matmul(
                    ps_o[:, h2, :],
                    lhsT=pT[:, kt2, h2, lq:lq + 128],
                    rhs=V_sb[:, pr, kt2, h2 * 65:(h2 + 1) * 65],
                    start=(j == 0),
                    stop=(j == nk - 1),
                )
        rc = rcp.tile([128, 2, 1], F32, tag="rc")
        nc.vector.reciprocal(out=rc, in_=ps_o[:, :, 64:65])
        ob = obp.tile([128, 2, 64], BF16, tag="ob")
        in0b, rcb = bass.broadcast_tensor_aps(ps_o[:, :, 0:64], rc)
        nc.vector.tensor_tensor(out=ob, in0=in0b, in1=rcb, op=mybir.AluOpType.mult)
        ps_t = sm.tile([128, 128], BF16, tag="sm", name="ps_t")
        nc.tensor.transpose(
            out=ps_t, in_=ob.rearrange("p h2 d -> p (h2 d)"), identity=ident
        )
        nc.vector.tensor_copy(out=OT_sb[:, pr, qt * 128:(qt + 1) * 128], in_=ps_t)

    def emit_o(c):
        """O-projection for one 512-column chunk of completed OT."""
        for ot in range(4):
            ps_f = sm.tile([128, 512], F32, tag="sm", name="ps_f")
            for pr2 in range(2):
                nc.tensor.matmul(
                    ps_f,
                    lhsT=wo_sb[:, pr2, ot * 128:(ot + 1) * 128],
                    rhs=OT_sb[:, pr2, c * 512:(c + 1) * 512],
                    start=(pr2 == 0),
                    stop=(pr2 == 1),
                )
            fs = fsp.tile([128, 512], BF16, tag="fs")
            nc.vector.tensor_copy(out=fs, in_=ps_f)
            nc.sync.dma_start(
                out=outT[ot * 128:(ot + 1) * 128, c * 512:(c + 1) * 512], in_=fs
            )

    # ---------------- pair 0: projections feed the band loop ----------------
    emit_qk_proj(False, 0, 0)
    emit_qk_proj(True, 0, 0)
    emit_qk_proj(True, 0, 1)
    emit_qk_proj(True, 0, 2)
    # pr1 Q/K chunks dribbled into the pr0 loop at odd kt
    pr1_pieces = [
        (False, 1, 0), (True, 1, 0), (True, 1, 1), (True, 1, 2),
        (False, 1, 1), (True, 1, 3), (False, 1, 2), (False, 1, 3),
    ]
    for kt in range(NT):
        emit_v(kt)
        if kt == 2:
            emit_qk_proj(False, 0, 1)
            emit_qk_proj(True, 0, 3)
        elif kt == 6:
            emit_qk_proj(False, 0, 2)
        elif kt == 10:
            emit_qk_proj(False, 0, 3)
        emit_scores(0, kt)
        emit_pv(0, kt)
        if kt % 2 == 1:
            emit_qk_proj(*pr1_pieces[kt // 2])

    # ---------------- pair 1: O-projection fills exp-wait bubbles -----------
    for kt in range(NT):
        emit_scores(1, kt)
        emit_pv(1, kt)
        if kt % 4 == 3:
            emit_o(kt // 4)


@functools.lru_cache(maxsize=2)
def _build(use_bias=True):
    nc = bacc.Bacc(
        "TRN2", target_bir_lowering=False, debug=False, num_devices=N_CORES
    )
    io = {
        "xT": nc.dram_tensor("xT", [D, S], BF16, kind="ExternalInput").ap(),
        "wq": nc.dram_tensor("wq", [D, 256], BF16, kind="ExternalInput").ap(),
        "wk": nc.dram_tensor("wk", [D, 256], BF16, kind="ExternalInput").ap(),
        "wv": nc.dram_tensor("wv", [D, 256], BF16, kind="ExternalInput").ap(),
        "wo": nc.dram_tensor("wo", [256, D], BF16, kind="ExternalInput").ap(),
        "bq": nc.dram_tensor("bq", [1, 256], BF16, kind="ExternalInput").ap(),
        "bk": nc.dram_tensor("bk", [1, 256], BF16, kind="ExternalInput").ap(),
        "bv": nc.dram_tensor("bv", [1, 256], BF16, kind="ExternalInput").ap(),
        "outT": nc.dram_tensor("outT", [D, S], BF16, kind="ExternalOutput").ap(),
    }
    with tile.TileContext(nc) as tc:
        with ExitStack() as ctx:
            _emit(ctx, tc, io, use_bias)
    nc.compile()
    return nc


def make_in_maps(x, W_Q, b_Q, W_K, b_K, W_V, b_V, W_O, b_O):
    in_maps = []
    for c in range(N_CORES):
        b, hg = c // 2, c % 2
        hs = hg * 256
        in_maps.append(
            {
                "xT": np.ascontiguousarray(x[b].T).astype(NBF),
                "wq": np.ascontiguousarray(W_Q[:, hs:hs + 256]).astype(NBF),
                "wk": np.ascontiguousarray(W_K[:, hs:hs + 256]).astype(NBF),
                "wv": np.ascontiguousarray(W_V[:, hs:hs + 256]).astype(NBF),
                "wo": np.ascontiguousarray(W_O[hs:hs + 256, :]).astype(NBF),
                "bq": b_Q[None, hs:hs + 256].astype(NBF),
                "bk": b_K[None, hs:hs + 256].astype(NBF),
                "bv": b_V[None, hs:hs + 256].astype(NBF),
            }
        )
    return in_maps


def kernel(x, W_Q, b_Q, W_K, b_K, W_V, b_V, W_O, b_O):
    global LAST_RESULTS
    x, W_Q, b_Q, W_K, b_K, W_V, b_V, W_O, b_O = (
        np.asarray(a, dtype=np.float32)
        for a in (x, W_Q, b_Q, W_K, b_K, W_V, b_V, W_O, b_O)
    )
    use_bias = bool(
        np.any(b_Q) or np.any(b_K) or np.any(b_V)
    )  # projection biases are all-zero in this model's inputs
    nc = _build(use_bias)
    in_maps = make_in_maps(x, W_Q, b_Q, W_K, b_K, W_V, b_V, W_O, b_O)
    res = run_bass_kernel_spmd(nc, in_maps, core_ids=list(range(N_CORES)))
    LAST_RESULTS = res
    out = np.empty((4, S, D), np.float32)
    for b in range(4):
        acc = res.results[2 * b]["outT"].astype(np.float32) + res.results[
            2 * b + 1
        ]["outT"].astype(np.float32)
        out[b] = acc.T + b_O[None, :]
    return out
